# revision 46
# baseline (speedup 1.0000x reference)
"""GAT (2-layer, PyG-style) on 8 Trainium2 NeuronCores via Bass/Tile.

v2 — engine-balanced rewrite of the dst-major graph-parallel design:

  - Nodes partitioned across 8 cores by dst id (6250 each); per core, own
    dsts are (degree, mustA)-sorted into 49 blocks of 128.  Each block is a
    [128 dst-partitions x (S_lo | S_hi)] slot grid; slot (d, s) holds one
    incoming edge of block-dst d.  Slot columns are split lo/hi because
    dma_gather indices are int16 (table split at row 32768, overlapping
    flex region balances the split per block optimally).
  - NO collective: each core computes the FULL node table (h | asrc) locally
    (phase A, batched matmuls over all 392 chunks of x^T), writes it to
    local DRAM, then phase B dma_gathers per-edge rows from it.
  - Phase B runs on "superblocks" (several 128-dst blocks per instruction)
    to amortize fixed instruction overheads:
       e   = asrc + adst            (DVE, f32)
       e   = lrelu(e)               (ACT, alpha=0.2)
       p   = exp(e)                 (ACT)
       pm  = p * mask               (DVE; mask kills padding slots)
       den = segsum(pm) + eps       (DVE reduce + fused add)
       al  = pm * (1/den)           (DVE -> bf16)
       alx = al replicated 16x      (ACT copy, bcast read)
       msg = h_gathered * alx       (DVE, packed bf16 2x mode)
       out = fold-tree sum of msg   (DVE, packed bf16 2x mode, in-place)
  - Gathers are per block (2 per block: lo/hi) with trailing -1 indices:
    the Q7 desc-gen drops trailing negatives, so each core only fetches its
    OWN grid size even though the program is SPMD-shared.
  - Layer 2's table rows are 256B; its table is stored partition-major
    (row(n) = (n%128)*392 + n//128) so phase-A writes are contiguous >=512B
    runs per partition (avoids the sub-512B DMA write penalty).

kernel(**inputs) takes FULL unsharded inputs, returns FULL [50000, 16] f32.
"""

import os
import sys

import numpy as np

sys.path.insert(0, "/opt/trn_rl_repo")

import concourse.bacc as bacc
import concourse.mybir as mybir
import concourse.tile as tile
from concourse.bass_utils import run_bass_kernel_spmd

F32 = mybir.dt.float32
BF16 = mybir.dt.bfloat16
I16 = mybir.dt.int16
AF = mybir.ActivationFunctionType
OP = mybir.AluOpType
AX = mybir.AxisListType

N = 50000
NC = 8
OWN = N // NC             # 6250
FIN = 128
HID = 16
HEADS = 8
FH1 = HEADS * HID         # 128
CLS = 16
NEG = 0.2
HALF = 32768
NPAD = 50176              # 392 * 128
BBASE = NPAD - HALF       # 17408
NCHUNK = NPAD // 128      # 392
BLKS = 49
OWNPAD = BLKS * 128       # 6272

# layer row layouts (f32 cols)
ROW1, A1OFF = 128, 120    # [h bf16 x128 | psum junk | asrc f32 x8 @120]
ROW2, A2OFF = 64, 63      # [h2 bf16 x16 | psum junk | asrc2 f32 @63]

TAILDROP = False          # runtime gather-count registers (crashes HW NEFF
                          # flow currently; full gathers instead)

CAP1 = 48                 # max slot columns per superblock, layer 1
CAP2 = 160
NBMAX = 8

MASKNEG = 0.0             # multiplicative mask

# dev-only ablation switches (timing experiments; break correctness)
ABL_NO_GATHER = os.environ.get("GAT_ABL_NO_GATHER", "0") == "1"
ABL_NO_MSG = os.environ.get("GAT_ABL_NO_MSG", "0") == "1"
ABL_NO_PHASEB = os.environ.get("GAT_ABL_NO_PHASEB", "0") == "1"


def _perm2(n):
    """L2 table row id for node n (partition-major)."""
    return (n % 128) * NCHUNK + n // 128


# ---------------------------------------------------------------- host prep

def _opt_lh(dg, A, F, B):
    """Min L+H with: forall d: A_d<=lo_d<=A_d+F_d, dg_d-lo_d<=H, lo_d<=L."""
    Lmin = int(A.max()) if len(A) else 0
    Hmin = int(B.max()) if len(B) else 0
    Dmax = int(dg.max()) if len(dg) else 0
    best = None
    for L in range(Lmin, max(Dmax, Lmin) + 1):
        H = max(int(np.max(dg - np.minimum(L, A + F), initial=0)), Hmin, 0)
        if best is None or L + H < best[0] + best[1]:
            best = (L, H)
        if H <= Hmin:
            break
    return best


def _group_sbs(S_LO, S_HI, cap, nbmax):
    """Greedy grouping of consecutive blocks into superblocks."""
    sbs = []
    j = 0
    while j < BLKS:
        nb = 1
        while j + nb < BLKS and nb < nbmax:
            slo = int(S_LO[j:j + nb + 1].max())
            shi = int(S_HI[j:j + nb + 1].max())
            tot = sum(int(S_LO[k]) + int(S_HI[k]) for k in range(j, j + nb + 1))
            if (nb + 1) * (slo + shi) > max(cap, slo + shi):
                break
            if (nb + 1) * (slo + shi) > 1.035 * tot + 4:
                break
            nb += 1
        sbs.append((j, nb, int(S_LO[j:j + nb].max()), int(S_HI[j:j + nb].max())))
        j += nb
    return sbs


def _wrap_idx(idx_pc):
    """[128 partition, COLS] int16 -> dma_gather idx tile [128, 8*COLS]."""
    P, C = idx_pc.shape
    assert P == 128
    flat = idx_pc.T.reshape(-1)            # position i = p + 128*c
    n16 = (len(flat) + 15) // 16
    t = np.zeros((16, n16), np.int16)
    t[np.arange(len(flat)) % 16, np.arange(len(flat)) // 16] = flat
    return np.tile(t, (8, 1))


def _prep(edge_index):
    ei = np.asarray(edge_index)
    loop = np.arange(N, dtype=np.int64)
    src_a = np.concatenate([ei[0].astype(np.int64), loop])
    dst_a = np.concatenate([ei[1].astype(np.int64), loop])

    # table rows are partition-major for both layers (contiguous phase-A
    # writes): node n lives at row (n%128)*NCHUNK + n//128
    def rowid(s, layer):
        return _perm2(s)

    # stratified dst->core assignment: global (deg, mustA) sort, dealt
    # round-robin so all cores see near-identical per-block degree profiles
    # (shared-max grids then cost ~nothing over per-core grids).
    gdeg = np.bincount(dst_a, minlength=N)
    grow = _perm2(src_a)
    gmustA = np.bincount(dst_a[grow < BBASE], minlength=N)
    gsigma = np.lexsort((-gmustA, -gdeg))        # global rank -> node id
    grank = np.empty(N, np.int64)
    grank[gsigma] = np.arange(N)
    # node d: core = grank[d] % NC, local rank = grank[d] // NC
    sig = []                                      # per-core local rank -> node
    for c in range(NC):
        sig.append(gsigma[np.arange(OWN) * NC + c])

    layers = {}
    core_base = []
    for c in range(NC):
        m = (grank[dst_a] % NC) == c
        s_c = src_a[m]
        d_c = grank[dst_a[m]] // NC               # local rank of dst
        deg = np.bincount(d_c, minlength=OWN)
        rank = np.arange(OWN)                     # already rank-ordered
        core_base.append(dict(s_c=s_c, d_c=d_c, deg=deg, rank=rank))

    for layer in (1, 2):
        L_all = np.zeros((NC, BLKS), np.int64)
        H_all = np.zeros((NC, BLKS), np.int64)
        pc = []
        for c in range(NC):
            cb = core_base[c]
            s_c, d_c, deg, rank = (cb["s_c"], cb["d_c"], cb["deg"],
                                   cb["rank"])
            r = rowid(s_c, layer)
            mustA = np.bincount(d_c[r < BBASE], minlength=OWN)
            flexc = np.bincount(d_c[(r >= BBASE) & (r < HALF)], minlength=OWN)
            mustB = deg - mustA - flexc
            degp = np.zeros(OWNPAD, np.int64); degp[:OWN] = deg
            Ap = np.zeros(OWNPAD, np.int64); Ap[:OWN] = mustA
            Fp = np.zeros(OWNPAD, np.int64); Fp[:OWN] = flexc
            Bp = np.zeros(OWNPAD, np.int64); Bp[:OWN] = mustB
            for j in range(BLKS):
                sl = slice(j * 128, (j + 1) * 128)
                L, H = _opt_lh(degp[sl], Ap[sl], Fp[sl], Bp[sl])
                L_all[c, j], H_all[c, j] = L, H
            pc.append(dict(mustA=mustA, flexc=flexc, r=r))

        # joint cross-core (L, H) choice per block: minimize
        # max_c(L_c) + max_c(H_c) over each core's feasibility frontier
        for j in range(BLKS):
            frontier = []          # per core: H_min(L) curve
            for c in range(NC):
                cb = core_base[c]
                lp = pc[c]
                sl = slice(j * 128, min((j + 1) * 128, OWN))
                dg = cb["deg"][sl]
                A = lp["mustA"][sl]
                F = lp["flexc"][sl]
                B = dg - A - F
                frontier.append((dg, A, F, B))
            Lmin = max(int(A.max()) if len(A) else 0
                       for (_, A, _, _) in frontier)
            Dmax = max(int(dg.max()) if len(dg) else 0
                       for (dg, _, _, _) in frontier)
            best = None
            for Lx in range(Lmin, max(Dmax, Lmin) + 1):
                Hs = []
                for (dg, A, F, B) in frontier:
                    Hmin = int(B.max()) if len(B) else 0
                    H = max(int(np.max(dg - np.minimum(Lx, A + F),
                                       initial=0)), Hmin, 0)
                    Hs.append(H)
                Hx = max(Hs)
                if best is None or Lx + Hx < best[0] + best[1]:
                    best = (Lx, Hx)
            Lj, Hj = best
            # each core re-derives its per-dst split against (Lj, Hj)
            L_all[:, j] = Lj
            H_all[:, j] = Hj

        S_LO = L_all.max(axis=0)
        S_HI = H_all.max(axis=0)
        cap = CAP1 if layer == 1 else CAP2
        sbs = _group_sbs(S_LO, S_HI, cap, NBMAX)

        # column layout: per sb [nb*slo | nb*shi]; idx arrays separate lo/hi
        col_of_blk_lo = np.zeros(BLKS, np.int64)
        col_of_blk_hi = np.zeros(BLKS, np.int64)
        iloff_of_blk = np.zeros(BLKS, np.int64)   # idx-space col offsets
        ihoff_of_blk = np.zeros(BLKS, np.int64)
        slo_of_blk = np.zeros(BLKS, np.int64)
        shi_of_blk = np.zeros(BLKS, np.int64)
        cols = 0
        ilo_cols = 0
        ihi_cols = 0
        for (j0, nb, slo, shi) in sbs:
            for b in range(nb):
                col_of_blk_lo[j0 + b] = cols + b * slo
                col_of_blk_hi[j0 + b] = cols + nb * slo + b * shi
                iloff_of_blk[j0 + b] = ilo_cols + b * slo
                ihoff_of_blk[j0 + b] = ihi_cols + b * shi
                slo_of_blk[j0 + b] = slo
                shi_of_blk[j0 + b] = shi
            cols += nb * (slo + shi)
            ilo_cols += nb * slo
            ihi_cols += nb * shi

        cores = []
        for c in range(NC):
            cb = core_base[c]
            s_c, d_c, deg = cb["s_c"], cb["d_c"], cb["deg"]
            lp = pc[c]
            mustA, flexc, r = lp["mustA"], lp["flexc"], lp["r"]
            rk = d_c
            blk = rk // 128
            prt = rk % 128
            Lc = L_all[c][blk]
            Hc = H_all[c][blk]
            # per-dst lo count (dst local rank d has block d//128)
            lo_t = np.maximum(mustA,
                              deg - H_all[c][np.arange(OWN) // 128])
            lo_t = np.minimum(lo_t, mustA + flexc)
            # per-edge flex rank within dst
            isflex = (r >= BBASE) & (r < HALF)
            keyf = d_c * 2 + (~isflex).astype(np.int64)
            of = np.argsort(keyf, kind="stable")
            ksf = keyf[of]
            _, fi, fc = np.unique(ksf, return_index=True, return_counts=True)
            frank_o = np.arange(len(ksf)) - np.repeat(fi, fc)
            frank = np.empty(len(s_c), np.int64)
            frank[of] = frank_o
            fa = lo_t - mustA                           # flex sent to lo
            is_lo = np.where(isflex, frank < fa[d_c], r < BBASE)
            # slot within (dst, half): appearance order
            key = rk * 2 + (~is_lo).astype(np.int64)
            order = np.argsort(key, kind="stable")
            ks = key[order]
            _, fi2, fc2 = np.unique(ks, return_index=True, return_counts=True)
            slot_o = np.arange(len(ks)) - np.repeat(fi2, fc2)
            slot = np.empty(len(s_c), np.int64)
            slot[order] = slot_o

            idx_lo = np.zeros((128, ilo_cols), np.int16)
            idx_hi = np.zeros((128, ihi_cols), np.int16)
            mask = np.zeros((128, cols), np.float32)
            mask[:] = -300.0            # additive pre-lrelu mask bias
            el = is_lo
            col_l = iloff_of_blk[blk[el]] + slot[el]
            idx_lo[prt[el], col_l] = r[el].astype(np.int16)
            mask[prt[el], col_of_blk_lo[blk[el]] + slot[el]] = 0.0
            eh = ~is_lo
            col_h = ihoff_of_blk[blk[eh]] + slot[eh]
            idx_hi[prt[eh], col_h] = (r[eh] - BBASE).astype(np.int16)
            mask[prt[eh], col_of_blk_hi[blk[eh]] + slot[eh]] = 0.0
            assert np.all(slot[el] < Lc[el]) and np.all(slot[eh] < Hc[eh])
            # columns >= per-core (L, H) are skipped at runtime via the
            # per-gather count register (idx stays 0: never read)
            cores.append(dict(
                idxlo=_wrap_idx(idx_lo) if ilo_cols else
                np.zeros((128, 8), np.int16),
                idxhi=_wrap_idx(idx_hi) if ihi_cols else
                np.zeros((128, 8), np.int16),
                mask=mask.astype(np.float32),
            ))

        # per-core runtime gather counts (emission order: per sb, per b,
        # lo then hi), in index units (multiples of 128)
        for c in range(NC):
            cnts = []
            for (j0, nb, slo, shi) in sbs:
                for b in range(nb):
                    if slo:
                        cnts.append(128 * max(int(L_all[c, j0 + b]), 1)
                                    if TAILDROP else 128 * slo)
                    if shi:
                        cnts.append(128 * max(int(H_all[c, j0 + b]), 1)
                                    if TAILDROP else 128 * shi)
            cores[c]["gcnt"] = np.array([cnts], np.int32)

        layers[layer] = dict(
            sbs=sbs, cols=cols, ilo_cols=ilo_cols, ihi_cols=ihi_cols,
            iloff=iloff_of_blk, ihoff=ihoff_of_blk,
            S_LO=S_LO, S_HI=S_HI, cores=cores, n_gath=len(cnts),
        )
    return layers, sig


# ------------------------------------------------------------- bass builder

def _build(lay, layer):
    sbs = lay["sbs"]
    cols = lay["cols"]
    iloff, ihoff = lay["iloff"], lay["ihoff"]
    ilo_cols, ihi_cols = max(lay["ilo_cols"], 1), max(lay["ihi_cols"], 1)

    if layer == 1:
        FH, AH, ROW, AOFF, FOUT = FH1, HEADS, ROW1, A1OFF, FH1
        WCOLS = FH + AH           # 136: [W | W@Asrc]
        CP0, CP1 = 72, 64         # psum[CP0:WCOLS] -> st[CP1:ROW] f32 copy
    else:
        FH, AH, ROW, AOFF, FOUT = CLS, 1, ROW2, A2OFF, CLS
        WCOLS = 64                # [W2 | zeros | W2@Asrc2 @63]
        CP0, CP1 = 8, 8
    ABN = 3 if layer == 1 else 7          # matmul chunks per psum tile
    ABI = 4                               # psum tiles per staged dma
    AB = ABN * ABI                        # chunks per phase-A iteration

    nc = bacc.Bacc("TRN2", target_bir_lowering=False, debug=False,
                   num_devices=NC, num_swdge_queues=2)
    xt = nc.declare_dram_parameter("xt", [128, NPAD], BF16, isOutput=False)
    xpermt = nc.declare_dram_parameter("xpermt", [128, OWNPAD], BF16,
                                       isOutput=False)
    wext = nc.declare_dram_parameter("wext", [128, WCOLS], BF16,
                                     isOutput=False)
    wadst = nc.declare_dram_parameter("wadst", [128, AH], BF16, isOutput=False)
    brow = nc.declare_dram_parameter("brow", [128, FOUT], F32, isOutput=False)
    idxlo = nc.declare_dram_parameter("idxlo", [128, 8 * ilo_cols], I16,
                                      isOutput=False)
    idxhi = nc.declare_dram_parameter("idxhi", [128, 8 * ihi_cols], I16,
                                      isOutput=False)
    maskp = nc.declare_dram_parameter("maskp", [128, cols], F32,
                                      isOutput=False)
    n_gath = max(lay["n_gath"], 1)
    gcnt = nc.declare_dram_parameter("gcnt", [1, n_gath], mybir.dt.int32,
                                     isOutput=False)
    out = nc.declare_dram_parameter("out", [OWNPAD, FOUT], F32, isOutput=True)
    th = nc.dram_tensor("th", [NPAD, ROW], F32)

    CAP = CAP1 if layer == 1 else CAP2

    with tile.TileContext(nc) as tc:
        with (
            tc.tile_pool(name="const", bufs=1) as cpool,
            tc.tile_pool(name="xa", bufs=3) as xpool,
            tc.tile_pool(name="stage", bufs=3) as spool,
            tc.tile_pool(name="psA", bufs=4, space="PSUM") as psA,
            tc.tile_pool(name="psB", bufs=2, space="PSUM") as psB,
            tc.tile_pool(name="gath", bufs=2) as gpool,
            tc.tile_pool(name="ep", bufs=2) as epool,
            tc.tile_pool(name="alx", bufs=1) as apool,
            tc.tile_pool(name="msg", bufs=1) as mpool,
            tc.tile_pool(name="fin", bufs=2) as fpool,
            tc.tile_pool(name="elu", bufs=1) as lpool,
        ):
            # constants
            w_sb = cpool.tile([128, WCOLS], BF16)
            nc.sync.dma_start(w_sb[:], wext[:])
            wa_sb = cpool.tile([128, AH], BF16)
            nc.sync.dma_start(wa_sb[:], wadst[:])
            b_sb = cpool.tile([128, FOUT], F32)
            nc.sync.dma_start(b_sb[:], brow[:])
            il_sb = cpool.tile([128, 8 * ilo_cols], I16)
            nc.sync.dma_start(il_sb[:], idxlo[:])
            ih_sb = cpool.tile([128, 8 * ihi_cols], I16)
            nc.sync.dma_start(ih_sb[:], idxhi[:])
            mk_sb = cpool.tile([128, cols], F32)
            nc.sync.dma_start(mk_sb[:], maskp[:])
            xp_sb = cpool.tile([128, OWNPAD], BF16)
            nc.sync.dma_start(xp_sb[:], xpermt[:])
            gc_sb = cpool.tile([1, n_gath], mybir.dt.int32)
            nc.sync.dma_start(gc_sb[:], gcnt[:])
            greg = nc.gpsimd.alloc_register("gcnt_reg")
            adst_all = cpool.tile([128, BLKS, AH], F32)

            # zero the gather pool buffers once (tail-dropped slots must read
            # finite data; later superblocks read older real rows, also fine)
            for _ in range(2 if TAILDROP else 0):
                gz = gpool.tile([128, CAP, ROW], F32, tag="g")
                nc.gpsimd.memset(gz[:], 0.0)

            # ---- phase A: full local table  th[n] = [h(n) bf16 | asrc f32]
            n_it = NCHUNK // AB
            rem = NCHUNK - n_it * AB
            for i in range(n_it + (1 if rem else 0)):
                nch = AB if i < n_it else rem
                npsum = (nch + ABN - 1) // ABN
                c0 = i * AB
                xt_t = xpool.tile([128, nch * 128], BF16, tag="xt")
                nc.sync.dma_start(xt_t[:],
                                  xt[:, c0 * 128:(c0 + nch) * 128])
                st = spool.tile([128, nch, ROW], F32, tag="st")
                for q in range(npsum):
                    k0 = q * ABN
                    kn = min(ABN, nch - k0)
                    ph = psA.tile([128, ABN, WCOLS], F32, tag="ph")
                    for t in range(kn):
                        nc.tensor.matmul(
                            ph[:, t, :],
                            xt_t[:, (k0 + t) * 128:(k0 + t + 1) * 128],
                            w_sb[:], start=True, stop=True)
                    nc.scalar.copy(
                        st.bitcast(BF16)[:, k0:k0 + kn, 0:FH],
                        ph[:, 0:kn, 0:FH])
                    # tail f32 cols incl asrc (+psum junk: initializes row)
                    nc.vector.tensor_copy(
                        st[:, k0:k0 + kn, CP1:ROW],
                        ph[:, 0:kn, CP0:WCOLS])
                nc.gpsimd.dma_start(
                    th[:, :].rearrange("(p b) r -> p b r", b=NCHUNK)
                    [:, c0:c0 + nch, :],
                    st[:, 0:nch, :])

            # adst for own dsts: [128, BLKS, AH]
            nbl = (BLKS + ABN - 1) // ABN
            for i in range(nbl):
                k0 = i * ABN
                kn = min(ABN, BLKS - k0)
                pa = psB.tile([128, ABN, AH], F32, tag="pa")
                for t in range(kn):
                    nc.tensor.matmul(
                        pa[:, t, :],
                        xp_sb[:, (k0 + t) * 128:(k0 + t + 1) * 128],
                        wa_sb[:], start=True, stop=True)
                nc.vector.tensor_copy(adst_all[:, k0:k0 + kn, :],
                                      pa[:, 0:kn, :])

            tc.strict_bb_all_engine_barrier()

            # ---- phase B: superblocks
            colbase = 0
            gidx = 0
            for (j0, nb, slo, shi) in ([] if ABL_NO_PHASEB else sbs):
                ncols = nb * (slo + shi)
                g = gpool.tile([128, ncols, ROW], F32, tag="g")
                if not ABL_NO_GATHER:
                    if slo:
                        nc.gpsimd.dma_gather(
                            g[:, 0:nb * slo, :], th[0:HALF, :],
                            il_sb[:, 8 * iloff[j0]:
                                  8 * (iloff[j0] + nb * slo)],
                            num_idxs=128 * nb * slo,
                            num_idxs_reg=128 * nb * slo,
                            elem_size=ROW, single_packet=False, queue_num=0)
                    if shi:
                        nc.gpsimd.dma_gather(
                            g[:, nb * slo:ncols, :],
                            th[BBASE:NPAD, :],
                            ih_sb[:, 8 * ihoff[j0]:
                                  8 * (ihoff[j0] + nb * shi)],
                            num_idxs=128 * nb * shi,
                            num_idxs_reg=128 * nb * shi,
                            elem_size=ROW, single_packet=False, queue_num=0)

                adst = adst_all[:, j0:j0 + nb, :]
                e = epool.tile([128, ncols, AH], F32, tag="e")
                lo_sl = slice(0, nb * slo)
                hi_sl = slice(nb * slo, ncols)
                if slo:
                    nc.vector.tensor_tensor(
                        e[:, lo_sl, :].rearrange("p (b s) h -> p b s h", b=nb),
                        g[:, lo_sl, AOFF:AOFF + AH]
                        .rearrange("p (b s) h -> p b s h", b=nb),
                        adst.unsqueeze(2).broadcast_to([128, nb, slo, AH]),
                        op=OP.add)
                if shi:
                    nc.vector.tensor_tensor(
                        e[:, hi_sl, :].rearrange("p (b s) h -> p b s h", b=nb),
                        g[:, hi_sl, AOFF:AOFF + AH]
                        .rearrange("p (b s) h -> p b s h", b=nb),
                        adst.unsqueeze(2).broadcast_to([128, nb, shi, AH]),
                        op=OP.add)
                # e += mask bias (-300 at padding slots, kills them pre-lrelu)
                nc.vector.tensor_tensor(
                    e[:], e[:],
                    mk_sb[:, colbase:colbase + ncols].unsqueeze(2)
                    .broadcast_to([128, ncols, AH]),
                    op=OP.add)
                e2 = epool.tile([128, ncols, AH], F32, tag="e2")
                nc.vector.scalar_tensor_tensor(
                    e2[:], e[:], NEG, e[:], op0=OP.mult, op1=OP.max)

                # alx[p, c, (h i)] = exp(e2[p, c, h])  (16-wide expand on ACT)
                nhid = HID if layer == 1 else CLS
                alx = apool.tile([128, ncols, FH], BF16, tag="alx")
                nc.scalar.activation(
                    alx[:].rearrange("p c (h i) -> p c h i", h=AH),
                    e2[:].unsqueeze(3)
                    .broadcast_to([128, ncols, AH, nhid]),
                    AF.Exp)

                denl = fpool.tile([128, nb, AH], F32, tag="denl")
                denh = fpool.tile([128, nb, AH], F32, tag="denh")
                alxh = alx[:].rearrange("p c (h i) -> p c h i", h=AH)
                if slo:
                    nc.vector.tensor_reduce(
                        denl[:],
                        alxh[:, lo_sl, :, 0]
                        .rearrange("p (b s) h -> p b h s", b=nb),
                        axis=AX.X, op=OP.add)
                if shi:
                    nc.vector.tensor_reduce(
                        denh[:],
                        alxh[:, hi_sl, :, 0]
                        .rearrange("p (b s) h -> p b h s", b=nb),
                        axis=AX.X, op=OP.add)
                den = fpool.tile([128, nb, AH], F32, tag="den")
                if slo and shi:
                    nc.vector.scalar_tensor_tensor(
                        den[:], denl[:], 1e-20, denh[:],
                        op0=OP.add, op1=OP.add)
                else:
                    nc.vector.tensor_scalar_add(
                        den[:], (denl if slo else denh)[:], 1e-20)
                rec = fpool.tile([128, nb, AH], F32, tag="rec")
                nc.vector.reciprocal(rec[:], den[:])

                msg = mpool.tile([128, ncols, FH], BF16, tag="msg")
                if not ABL_NO_MSG:
                    nc.vector.tensor_tensor(
                        msg[:], g.bitcast(BF16)[:, :, 0:FH], alx[:],
                        op=OP.mult)

                    # fold-tree slot sums (in place) per block, lo/hi regions
                    for (sl0, ns) in ((0, slo), (nb * slo, shi)):
                        s = ns
                        while s > 1:
                            k = s // 2
                            v = msg[:, sl0:sl0 + nb * ns, :].rearrange(
                                "p (b s) f -> p b s f", b=nb)
                            nc.vector.tensor_tensor(
                                v[:, :, 0:k, :], v[:, :, 0:k, :],
                                v[:, :, s - k:s, :], op=OP.add)
                            s = s - k
                outun = fpool.tile([128, nb, FH], BF16, tag="outun")
                if slo and shi:
                    mlo0 = msg[:, 0:nb * slo, :].rearrange(
                        "p (b s) f -> p b s f", b=nb)[:, :, 0, :]
                    mhi0 = msg[:, nb * slo:, :].rearrange(
                        "p (b s) f -> p b s f", b=nb)[:, :, 0, :]
                    nc.vector.tensor_tensor(outun[:], mlo0, mhi0, op=OP.add)
                elif slo:
                    mlo0 = msg[:, 0:nb * slo, :].rearrange(
                        "p (b s) f -> p b s f", b=nb)[:, :, 0, :]
                    nc.vector.tensor_copy(outun[:], mlo0)
                else:
                    mhi0 = msg[:, nb * slo:, :].rearrange(
                        "p (b s) f -> p b s f", b=nb)[:, :, 0, :]
                    nc.vector.tensor_copy(outun[:], mhi0)

                # normalize by 1/den, add bias
                tf = lpool.tile([128, nb, FOUT], F32, tag="tf")
                nc.vector.tensor_tensor(
                    tf[:].rearrange("p b (h i) -> p b h i", h=AH),
                    outun[:].rearrange("p b (h i) -> p b h i", h=AH),
                    rec[:].unsqueeze(3).broadcast_to([128, nb, AH, nhid]),
                    op=OP.mult)
                fin = fpool.tile([128, nb, FOUT], F32, tag="fin")
                if layer == 1:
                    nc.vector.tensor_tensor(
                        tf[:], tf[:],
                        b_sb[:].unsqueeze(1).broadcast_to([128, nb, FOUT]),
                        op=OP.add)
                    mn = lpool.tile([128, nb, FOUT], F32, tag="mn")
                    nc.vector.tensor_scalar_min(mn[:], tf[:], 0.0)
                    ex = lpool.tile([128, nb, FOUT], F32, tag="ex")
                    nc.scalar.activation(ex[:], mn[:], AF.Exp)
                    mx = lpool.tile([128, nb, FOUT], F32, tag="mx")
                    nc.vector.tensor_scalar_max(mx[:], tf[:], 0.0)
                    nc.vector.scalar_tensor_tensor(
                        fin[:], ex[:], -1.0, mx[:], op0=OP.add, op1=OP.add)
                else:
                    nc.vector.tensor_tensor(
                        fin[:], tf[:],
                        b_sb[:].unsqueeze(1).broadcast_to([128, nb, FOUT]),
                        op=OP.add)
                nc.sync.dma_start(
                    out[j0 * 128:(j0 + nb) * 128, :]
                    .rearrange("(b p) f -> p b f", p=128),
                    fin[:])
                colbase += ncols

    nc.compile()
    return nc


# --------------------------------------------------------------- execution

_CACHE = {}
TRACE = os.environ.get("GAT_TRACE", "0") == "1"
RUN_KW = {}


def _to_bf16(a):
    return np.asarray(a, np.float32).astype(mybir.dt.np(BF16))


def _amat(att, fh, hid, heads):
    m = np.zeros((fh, heads), np.float32)
    for h in range(heads):
        m[h * hid:(h + 1) * hid, h] = att[h]
    return m


def kernel(x, edge_index, W1, att_src1, att_dst1, b1, W2, att_src2, att_dst2,
           b2):
    x = np.asarray(x, np.float32)
    ei = np.asarray(edge_index)
    if "prep" not in _CACHE:
        _CACHE["prep"] = _prep(ei)
    layers, sig = _CACHE["prep"]

    if "nc1" not in _CACHE:
        _CACHE["nc1"] = _build(layers[1], 1)
        _CACHE["nc2"] = _build(layers[2], 2)
    nc1, nc2 = _CACHE["nc1"], _CACHE["nc2"]

    # ---- layer 1
    W1 = np.asarray(W1, np.float32)
    As1 = _amat(np.asarray(att_src1, np.float32), FH1, HID, HEADS)
    Ad1 = _amat(np.asarray(att_dst1, np.float32), FH1, HID, HEADS)
    w1ext = _to_bf16(np.concatenate([W1, W1 @ As1], axis=1))
    w1adst = _to_bf16(W1 @ Ad1)
    b1row = np.tile(np.asarray(b1, np.float32)[None, :], (128, 1))

    xpad = np.zeros((NPAD, FIN), np.float32)
    xpad[:N] = x
    xt = _to_bf16(xpad.T.copy())

    lay1 = layers[1]
    in_maps = []
    for c in range(NC):
        xperm = np.zeros((OWNPAD, FIN), np.float32)
        xperm[:OWN] = x[sig[c]]
        in_maps.append(dict(
            xt=xt, wext=w1ext, wadst=w1adst, brow=b1row,
            xpermt=_to_bf16(xperm.T.copy()),
            idxlo=lay1["cores"][c]["idxlo"],
            idxhi=lay1["cores"][c]["idxhi"],
            maskp=lay1["cores"][c]["mask"],
            gcnt=lay1["cores"][c]["gcnt"],
        ))
    res1 = run_bass_kernel_spmd(nc1, in_maps, list(range(NC)),
                                trace=TRACE, **RUN_KW)

    x2 = np.zeros((N, FH1), np.float32)
    for c in range(NC):
        x2[sig[c]] = res1.results[c]["out"][:OWN]

    # ---- layer 2 (table rows permuted partition-major)
    W2 = np.asarray(W2, np.float32)
    As2 = _amat(np.asarray(att_src2, np.float32), CLS, CLS, 1)
    Ad2 = _amat(np.asarray(att_dst2, np.float32), CLS, CLS, 1)
    w2ext = _to_bf16(np.concatenate(
        [W2, np.zeros((FH1, 64 - CLS - 1), np.float32), W2 @ As2], axis=1))
    w2adst = _to_bf16(W2 @ Ad2)
    b2row = np.tile(np.asarray(b2, np.float32)[None, :], (128, 1))

    # xt2 column n must hold the node whose TABLE row is ... phase A writes
    # node (chunk c, partition p) to row p*NCHUNK+c; we want row perm2(n) =
    # (n%128)*NCHUNK + n//128, i.e. p = n%128, c = n//128 -> xt2 col
    # (c*128+p) = node n: same layout as layer 1.
    x2pad = np.zeros((NPAD, FH1), np.float32)
    x2pad[:N] = x2
    x2t = _to_bf16(x2pad.T.copy())

    lay2 = layers[2]
    in_maps2 = []
    for c in range(NC):
        xperm = np.zeros((OWNPAD, FH1), np.float32)
        xperm[:OWN] = x2[sig[c]]
        in_maps2.append(dict(
            xt=x2t, wext=w2ext, wadst=w2adst, brow=b2row,
            xpermt=_to_bf16(xperm.T.copy()),
            idxlo=lay2["cores"][c]["idxlo"],
            idxhi=lay2["cores"][c]["idxhi"],
            maskp=lay2["cores"][c]["mask"],
            gcnt=lay2["cores"][c]["gcnt"],
        ))
    res2 = run_bass_kernel_spmd(nc2, in_maps2, list(range(NC)),
                                trace=TRACE, **RUN_KW)

    outf = np.zeros((N, CLS), np.float32)
    for c in range(NC):
        outf[sig[c]] = res2.results[c]["out"][:OWN]
    kernel.last_results = (res1, res2)
    return outf
